# revision 8
# baseline (speedup 1.0000x reference)
"""Malvar-He-Cutler demosaic on 8 Trainium2 NeuronCores.

Strategy (W-sharding, all ops local per core):
  - Full input x [4096, 6144] f32 is reflect-padded on host and column-
    sharded into 8 slices of 768 cols (+2 halo each side) -> xp [4100, 772].
  - Per core, row tiles of 124 output rows. Input tile X [128, 772] is
    loaded parity-permuted (partitions 0-63 = even image rows, 64-127 =
    odd rows) by two strided DMAs; the banded stationary matrices absorb
    the permutation.
  - The 4 needed conv maps (2 per pixel: R/B sites need G-interp + the
    opposite-color kernel, G sites need the two R/B kernels) are computed
    as 4 matmul groups x 5 column-tap passes, accumulating in PSUM.
    Each group packs two 62-row conv maps at partition bases 0 and 64
    (M=128). Moving operand = stride-2 column slice of X (one column
    parity per group), dtype float32r for 1 cycle/row PE throughput.
  - DVE tensor_scalar(min 1.0, max 0.0) copies PSUM -> RGB-interleaved
    assembly buffer (fused clip). The x-passthrough channel values are
    copied by ACT/GPSIMD from a partition-shifted copy of X (engine APs
    require partition base 0/32/64, so a SBUF->SBUF DMA re-bases first).
  - Strided DMAs write even/odd assembled rows to the output shard
    [4096, 768*3]; host concatenates shards along W.
"""

import numpy as np

H, W = 4096, 6144
NCORES = 8
CS = W // NCORES          # 768 cols per core
TILE_R = 124              # output rows per tile
NC2 = CS // 2             # matmul moving free dim (384)

_PROGRAM = None


def _f32r_supported():
    return True


def _build_program(use_f32r=True, h=H, cs=CS):
    from concourse import bacc, mybir, tile

    f32 = mybir.dt.float32
    mmdt = mybir.dt.float32r if use_f32r else mybir.dt.float32
    CS, NC2 = cs, cs // 2  # noqa: shadow module constants intentionally

    nc = bacc.Bacc(None, target_bir_lowering=False, debug=True)
    xp_d = nc.dram_tensor("xp", [h + 4, CS + 4], f32, kind="ExternalInput")
    w_d = nc.dram_tensor("wst", [128, 20 * 128], f32, kind="ExternalInput")
    out_d = nc.dram_tensor("out", [h, CS * 3], f32, kind="ExternalOutput")

    r0s = [TILE_R * i for i in range(h // TILE_R)]
    if r0s[-1] + TILE_R < h:
        r0s.append(h - TILE_R)

    mn, mx = mybir.AluOpType.min, mybir.AluOpType.max
    copy_f = mybir.ActivationFunctionType.Copy

    with tile.TileContext(nc) as tc:
        with tc.tile_pool(name="wpool", bufs=1) as wpool, \
             tc.tile_pool(name="xpool", bufs=3) as xpool, \
             tc.tile_pool(name="cpool", bufs=2) as cpool, \
             tc.tile_pool(name="apool", bufs=2) as apool, \
             tc.tile_pool(name="ppool", bufs=2, space="PSUM") as ppool:

            wt = wpool.tile([128, 20 * 128], mmdt, name="wt")
            nc.sync.dma_start(out=wt[:], in_=w_d.ap().bitcast(mmdt))

            for r0 in r0s:
                X = xpool.tile([128, CS + 4], mmdt, name="X", tag="X")
                # one DMA, parity-permuted rows: partition p<64 <- row r0+2p,
                # p>=64 <- row r0+2(p-64)+1
                nc.sync.dma_start(
                    out=X[0:128, :],
                    in_=xp_d[r0 : r0 + 128, :].rearrange("(a t) c -> t a c", t=2).bitcast(mmdt),
                )
                # partition re-base for the x-passthrough copies
                xc = cpool.tile([128, CS + 4], f32, name="xc", tag="xc")
                nc.sync.dma_start(out=xc[0:62, :], in_=X[1:63, :].bitcast(f32))
                nc.sync.dma_start(out=xc[64:126, :], in_=X[65:127, :].bitcast(f32))

                Xr = X
                psums = []
                for g in range(4):
                    ps = ppool.tile([128, NC2], f32, name=f"ps{g}", tag=f"ps{g}")
                    coff = 0 if g < 2 else 1
                    for dj in range(5):
                        mov = Xr[:, coff + dj : coff + dj + CS - 1 : 2]
                        nc.tensor.matmul(
                            ps[:],
                            lhsT=wt[:, (g * 5 + dj) * 128 : (g * 5 + dj + 1) * 128],
                            rhs=mov,
                            start=(dj == 0),
                            stop=(dj == 4),
                        )
                    psums.append(ps)
                E1, E2, O1, O2 = psums

                asm = apool.tile([128, CS * 3], f32, name="asm", tag="asm")

                def clip(o, i):
                    nc.vector.tensor_scalar(o, i, 1.0, 0.0, op0=mn, op1=mx)

                L = CS * 3
                clip(asm[0:62, 1:L:6], E1[0:62, :])      # G @ (e,e)
                clip(asm[64:126, 0:L:6], E1[64:126, :])  # R @ (o,e)
                clip(asm[0:128, 2:L:6], E2[0:128, :])    # B @ (e,e)+(o,e)
                clip(asm[0:128, 3:L:6], O1[0:128, :])    # R @ (e,o)+(o,o)
                clip(asm[0:62, 5:L:6], O2[0:62, :])      # B @ (e,o)
                clip(asm[64:126, 4:L:6], O2[64:126, :])  # G @ (o,o)

                # x passthrough (no clip needed: x in [0,1))
                nc.scalar.activation(asm[0:62, 0:L:6], xc[0:62, 2 : 2 + CS : 2], copy_f)
                nc.scalar.activation(asm[64:126, 1:L:6], xc[64:126, 2 : 2 + CS : 2], copy_f)
                nc.gpsimd.tensor_copy(asm[0:62, 4:L:6], xc[0:62, 3 : 3 + CS : 2])
                nc.gpsimd.tensor_copy(asm[64:126, 5:L:6], xc[64:126, 3 : 3 + CS : 2])

                # stores ride the ACT HWDGE ring so they never head-of-line
                # block the next tile's loads on the SP ring
                if r0 % TILE_R == 0:
                    nc.scalar.dma_start(out=out_d[r0 : r0 + TILE_R : 2, :], in_=asm[0:62, :])
                    nc.scalar.dma_start(out=out_d[r0 + 1 : r0 + TILE_R : 2, :], in_=asm[64:126, :])
                else:
                    # overlap tile: emit only the rows no earlier tile wrote
                    new0 = (r0s[-2] + TILE_R - r0) // 2  # first new slot
                    nc.scalar.dma_start(
                        out=out_d[r0 + 2 * new0 : r0 + TILE_R : 2, :],
                        in_=asm[new0:62, :],
                    )
                    nc.scalar.dma_start(
                        out=out_d[r0 + 2 * new0 + 1 : r0 + TILE_R : 2, :],
                        in_=asm[64 + new0 : 126, :],
                    )
    nc.compile()
    return nc


def _get_program():
    global _PROGRAM
    if _PROGRAM is None:
        _PROGRAM = _build_program()
    return _PROGRAM


def _build_stationary(kern):
    """kern: [4,5,5] f32 -> W [128, 20*128] f32 (SBUF layout, lhsT per slice)."""
    groups = [(0, 2), (3, 1), (1, 3), (2, 0)]  # (even-row kernel, odd-row kernel)
    Wm = np.zeros((20, 128, 128), np.float32)
    t = np.arange(62)
    for g, (ka, kb) in enumerate(groups):
        for dj in range(5):
            Wq = Wm[g * 5 + dj]
            Wq[t, t] += kern[ka, 0, dj]
            Wq[64 + t, t] += kern[ka, 1, dj]
            Wq[t + 1, t] += kern[ka, 2, dj]
            Wq[65 + t, t] += kern[ka, 3, dj]
            Wq[t + 2, t] += kern[ka, 4, dj]
            Wq[64 + t, 64 + t] += kern[kb, 0, dj]
            Wq[t + 1, 64 + t] += kern[kb, 1, dj]
            Wq[65 + t, 64 + t] += kern[kb, 2, dj]
            Wq[t + 2, 64 + t] += kern[kb, 3, dj]
            Wq[66 + t, 64 + t] += kern[kb, 4, dj]
    # [20,128p,128m] -> [128p, 20*128]
    return np.ascontiguousarray(Wm.transpose(1, 0, 2).reshape(128, 20 * 128))


def kernel(x, kernels, _trace=False):
    from concourse.bass_utils import run_bass_kernel_spmd

    x = np.asarray(x, dtype=np.float32)
    kern = np.asarray(kernels, dtype=np.float32).reshape(4, 5, 5)
    wst = _build_stationary(kern)
    xpad = np.pad(x, 2, mode="reflect")

    in_maps = []
    for c in range(NCORES):
        shard = np.ascontiguousarray(xpad[:, c * CS : c * CS + CS + 4])
        in_maps.append({"xp": shard, "wst": wst})

    nc = _get_program()
    res = run_bass_kernel_spmd(nc, in_maps, list(range(NCORES)), trace=_trace)
    out = np.concatenate(
        [res.results[c]["out"].reshape(H, CS, 3) for c in range(NCORES)], axis=1
    )
    if _trace:
        return out, res
    return out


# revision 11
# speedup vs baseline: 1.3561x; 1.3561x over previous
"""Malvar-He-Cutler demosaic on 8 Trainium2 NeuronCores.

Strategy (W-sharding, all ops local per core):
  - Full input x [4096, 6144] f32 is reflect-padded on host and column-
    sharded into 8 slices of 768 cols (+2 halo each side) -> xp [4100, 772].
  - Per core, row tiles of 124 output rows. Input tile X [128, 772] is
    loaded parity-permuted (partitions 0-63 = even image rows, 64-127 =
    odd rows) by two strided DMAs; the banded stationary matrices absorb
    the permutation.
  - The 4 needed conv maps (2 per pixel: R/B sites need G-interp + the
    opposite-color kernel, G sites need the two R/B kernels) are computed
    as 4 matmul groups x 5 column-tap passes, accumulating in PSUM.
    Each group packs two 62-row conv maps at partition bases 0 and 64
    (M=128). Moving operand = stride-2 column slice of X (one column
    parity per group), dtype float32r for 1 cycle/row PE throughput.
  - DVE tensor_scalar(min 1.0, max 0.0) copies PSUM -> RGB-interleaved
    assembly buffer (fused clip). The x-passthrough channel values are
    copied by ACT/GPSIMD from a partition-shifted copy of X (engine APs
    require partition base 0/32/64, so a SBUF->SBUF DMA re-bases first).
  - Strided DMAs write even/odd assembled rows to the output shard
    [4096, 768*3]; host concatenates shards along W.
"""

import numpy as np

H, W = 4096, 6144
NCORES = 8
CS = W // NCORES          # 768 cols per core
TILE_R = 124              # output rows per tile
NC2 = CS // 2             # matmul moving free dim (384)

_PROGRAM = None


def _f32r_supported():
    return True


def _build_program(use_f32r=True, h=H, cs=CS):
    from concourse import bacc, mybir, tile

    f32 = mybir.dt.float32
    mmdt = mybir.dt.float32r if use_f32r else mybir.dt.float32
    CS, NC2 = cs, cs // 2  # noqa: shadow module constants intentionally

    nc = bacc.Bacc(None, target_bir_lowering=False, debug=True)
    xp_d = nc.dram_tensor("xp", [h + 4, CS + 4], f32, kind="ExternalInput")
    w_d = nc.dram_tensor("wst", [128, 20 * 128], f32, kind="ExternalInput")
    out_d = nc.dram_tensor("out", [h, CS * 3], f32, kind="ExternalOutput")

    r0s = [TILE_R * i for i in range(h // TILE_R)]
    if r0s[-1] + TILE_R < h:
        r0s.append(h - TILE_R)

    mn, mx = mybir.AluOpType.min, mybir.AluOpType.max
    copy_f = mybir.ActivationFunctionType.Copy

    STORE_SKEW = 2  # store tile i while computing tile i+2

    with tile.TileContext(nc) as tc:
        with tc.tile_pool(name="wpool", bufs=1) as wpool, \
             tc.tile_pool(name="xpool", bufs=3) as xpool, \
             tc.tile_pool(name="cpool", bufs=2) as cpool, \
             tc.tile_pool(name="apool", bufs=STORE_SKEW + 2) as apool, \
             tc.tile_pool(name="ppool", bufs=2, space="PSUM") as ppool:

            wt = wpool.tile([128, 20 * 128], mmdt, name="wt")
            nc.sync.dma_start(out=wt[:], in_=w_d.ap().bitcast(mmdt))

            def store(r0, asm):
                # stores on the ACT HWDGE ring, issued STORE_SKEW tiles late so
                # their semaphore waits are already satisfied at issue time
                if r0 % TILE_R == 0:
                    nc.scalar.dma_start(out=out_d[r0 : r0 + TILE_R : 2, :], in_=asm[0:62, :])
                    nc.scalar.dma_start(out=out_d[r0 + 1 : r0 + TILE_R : 2, :], in_=asm[64:126, :])
                else:
                    # overlap tile: emit only the rows no earlier tile wrote
                    new0 = (r0s[-2] + TILE_R - r0) // 2  # first new slot
                    nc.scalar.dma_start(
                        out=out_d[r0 + 2 * new0 : r0 + TILE_R : 2, :],
                        in_=asm[new0:62, :],
                    )
                    nc.scalar.dma_start(
                        out=out_d[r0 + 2 * new0 + 1 : r0 + TILE_R : 2, :],
                        in_=asm[64 + new0 : 126, :],
                    )

            pending = []
            for r0 in r0s:
                X = xpool.tile([128, CS + 4], mmdt, name="X", tag="X")
                # natural row order: partition p <- xp row r0+p (contiguous)
                nc.sync.dma_start(out=X[:], in_=xp_d[r0 : r0 + 128, :].bitcast(mmdt))
                # center rows for the x-passthrough copies, re-read from DRAM
                # (engine APs need partition base 0/32/64, so X can't serve
                # directly; a DRAM re-read beats a partition-strided SBUF DMA)
                xc = cpool.tile([128, CS + 4], f32, name="xc", tag="xc")
                nc.sync.dma_start(out=xc[0:62, :], in_=xp_d[r0 + 2 : r0 + 126 : 2, :])
                nc.sync.dma_start(out=xc[64:126, :], in_=xp_d[r0 + 3 : r0 + 127 : 2, :])

                psums = []
                for g in range(4):
                    ps = ppool.tile([128, NC2], f32, name=f"ps{g}", tag=f"ps{g}")
                    coff = 0 if g < 2 else 1
                    for dj in range(5):
                        mov = X[:, coff + dj : coff + dj + CS - 1 : 2]
                        nc.tensor.matmul(
                            ps[:],
                            lhsT=wt[:, (g * 5 + dj) * 128 : (g * 5 + dj + 1) * 128],
                            rhs=mov,
                            start=(dj == 0),
                            stop=(dj == 4),
                        )
                    psums.append(ps)
                E1, E2, O1, O2 = psums

                asm = apool.tile([128, CS * 3], f32, name="asm", tag="asm")

                def clip(o, i):
                    nc.vector.tensor_scalar(o, i, 1.0, 0.0, op0=mn, op1=mx)

                L = CS * 3
                clip(asm[0:62, 1:L:6], E1[0:62, :])      # G @ (e,e)
                clip(asm[64:126, 0:L:6], E1[64:126, :])  # R @ (o,e)
                clip(asm[0:128, 2:L:6], E2[0:128, :])    # B @ (e,e)+(o,e)
                clip(asm[0:128, 3:L:6], O1[0:128, :])    # R @ (e,o)+(o,o)
                clip(asm[0:62, 5:L:6], O2[0:62, :])      # B @ (e,o)
                clip(asm[64:126, 4:L:6], O2[64:126, :])  # G @ (o,o)

                # x passthrough (no clip needed: x in [0,1))
                nc.scalar.activation(asm[0:62, 0:L:6], xc[0:62, 2 : 2 + CS : 2], copy_f)
                nc.scalar.activation(asm[64:126, 1:L:6], xc[64:126, 2 : 2 + CS : 2], copy_f)
                nc.gpsimd.tensor_copy(asm[0:62, 4:L:6], xc[0:62, 3 : 3 + CS : 2])
                nc.gpsimd.tensor_copy(asm[64:126, 5:L:6], xc[64:126, 3 : 3 + CS : 2])

                pending.append((r0, asm))
                if len(pending) > STORE_SKEW:
                    store(*pending.pop(0))
            for item in pending:
                store(*item)
    nc.compile()
    return nc


def _get_program():
    global _PROGRAM
    if _PROGRAM is None:
        _PROGRAM = _build_program()
    return _PROGRAM


def _build_stationary(kern):
    """kern: [4,5,5] f32 -> W [128, 20*128] f32 (SBUF layout, lhsT per slice)."""
    groups = [(0, 2), (3, 1), (1, 3), (2, 0)]  # (even-row kernel, odd-row kernel)
    Wm = np.zeros((20, 128, 128), np.float32)
    t = np.arange(62)
    for g, (ka, kb) in enumerate(groups):
        for dj in range(5):
            Wq = Wm[g * 5 + dj]
            for di in range(5):
                # X row order is natural: partition p = xp row r0+p
                Wq[2 * t + di, t] += kern[ka, di, dj]          # even out rows
                Wq[2 * t + 1 + di, 64 + t] += kern[kb, di, dj]  # odd out rows
    # [20,128p,128m] -> [128p, 20*128]
    return np.ascontiguousarray(Wm.transpose(1, 0, 2).reshape(128, 20 * 128))


def kernel(x, kernels, _trace=False):
    from concourse.bass_utils import run_bass_kernel_spmd

    x = np.asarray(x, dtype=np.float32)
    kern = np.asarray(kernels, dtype=np.float32).reshape(4, 5, 5)
    wst = _build_stationary(kern)
    xpad = np.pad(x, 2, mode="reflect")

    in_maps = []
    for c in range(NCORES):
        shard = np.ascontiguousarray(xpad[:, c * CS : c * CS + CS + 4])
        in_maps.append({"xp": shard, "wst": wst})

    nc = _get_program()
    res = run_bass_kernel_spmd(nc, in_maps, list(range(NCORES)), trace=_trace)
    out = np.concatenate(
        [res.results[c]["out"].reshape(H, CS, 3) for c in range(NCORES)], axis=1
    )
    if _trace:
        return out, res
    return out


# revision 15
# speedup vs baseline: 1.7113x; 1.2619x over previous
"""Malvar-He-Cutler demosaic on 8 Trainium2 NeuronCores.

Strategy (W-sharding, all ops local per core):
  - Full input x [4096, 6144] f32 is reflect-padded on host and column-
    sharded into 8 slices of 768 cols (+2 halo each side) -> xp [4100, 772].
  - Per core, row tiles of 124 output rows. Input tile X [128, 772] is
    loaded parity-permuted (partitions 0-63 = even image rows, 64-127 =
    odd rows) by two strided DMAs; the banded stationary matrices absorb
    the permutation.
  - The 4 needed conv maps (2 per pixel: R/B sites need G-interp + the
    opposite-color kernel, G sites need the two R/B kernels) are computed
    as 4 matmul groups x 5 column-tap passes, accumulating in PSUM.
    Each group packs two 62-row conv maps at partition bases 0 and 64
    (M=128). Moving operand = stride-2 column slice of X (one column
    parity per group), dtype float32r for 1 cycle/row PE throughput.
  - DVE tensor_scalar(min 1.0, max 0.0) copies PSUM -> RGB-interleaved
    assembly buffer (fused clip). The x-passthrough channel values are
    copied by ACT/GPSIMD from a partition-shifted copy of X (engine APs
    require partition base 0/32/64, so a SBUF->SBUF DMA re-bases first).
  - Strided DMAs write even/odd assembled rows to the output shard
    [4096, 768*3]; host concatenates shards along W.
"""

import numpy as np

H, W = 4096, 6144
NCORES = 8
CS = W // NCORES          # 768 cols per core
TILE_R = 124              # output rows per tile
NC2 = CS // 2             # matmul moving free dim (384)

_PROGRAM = None


def _f32r_supported():
    return True


def _build_program(use_f32r=True, h=H, cs=CS):
    from concourse import bacc, mybir, tile

    f32 = mybir.dt.float32
    mmdt = mybir.dt.float32r if use_f32r else mybir.dt.float32
    CS, NC2 = cs, cs // 2  # noqa: shadow module constants intentionally

    nc = bacc.Bacc(None, target_bir_lowering=False, debug=True)
    xp_d = nc.dram_tensor("xp", [h + 4, CS + 4], f32, kind="ExternalInput")
    w_d = nc.dram_tensor("wst", [128, 20 * 128], f32, kind="ExternalInput")
    out_d = nc.dram_tensor("out", [h, CS * 3], f32, kind="ExternalOutput")

    r0s = [TILE_R * i for i in range(h // TILE_R)]
    if r0s[-1] + TILE_R < h:
        r0s.append(h - TILE_R)

    mn, mx = mybir.AluOpType.min, mybir.AluOpType.max
    copy_f = mybir.ActivationFunctionType.Copy

    STORE_SKEW = 2  # store tile i while computing tile i+2

    with tile.TileContext(nc) as tc:
        with tc.tile_pool(name="wpool", bufs=1) as wpool, \
             tc.tile_pool(name="xpool", bufs=4) as xpool, \
             tc.tile_pool(name="cpool", bufs=4) as cpool, \
             tc.tile_pool(name="apool", bufs=STORE_SKEW + 2) as apool, \
             tc.tile_pool(name="ppool", bufs=2, space="PSUM") as ppool:

            wt = wpool.tile([128, 20 * 128], mmdt, name="wt")
            nc.sync.dma_start(out=wt[:], in_=w_d.ap().bitcast(mmdt))

            def store(r0, asm):
                # stores on the ACT HWDGE ring, issued STORE_SKEW tiles late so
                # their semaphore waits are already satisfied at issue time
                if r0 % TILE_R == 0:
                    nc.scalar.dma_start(out=out_d[r0 : r0 + TILE_R : 2, :], in_=asm[0:62, :])
                    nc.scalar.dma_start(out=out_d[r0 + 1 : r0 + TILE_R : 2, :], in_=asm[64:126, :])
                else:
                    # overlap tile: emit only the rows no earlier tile wrote
                    new0 = (r0s[-2] + TILE_R - r0) // 2  # first new slot
                    nc.scalar.dma_start(
                        out=out_d[r0 + 2 * new0 : r0 + TILE_R : 2, :],
                        in_=asm[new0:62, :],
                    )
                    nc.scalar.dma_start(
                        out=out_d[r0 + 2 * new0 + 1 : r0 + TILE_R : 2, :],
                        in_=asm[64 + new0 : 126, :],
                    )

            LOAD_AHEAD = 2

            def issue_loads(r0):
                X = xpool.tile([128, CS + 4], mmdt, name="X", tag="X")
                # natural row order: partition p <- xp row r0+p (contiguous).
                # SWDGE (gpsimd) splits one DMA across all 16 SDMA engines;
                # the SP HWDGE ring funnels into only 2 and bottlenecks.
                nc.gpsimd.dma_start(out=X[:], in_=xp_d[r0 : r0 + 128, :].bitcast(mmdt))
                # center rows for the x-passthrough copies, re-read from DRAM
                # (engine APs need partition base 0/32/64, so X can't serve
                # directly). One DMA: dst partitions {0-61, 64-125}.
                xc = cpool.tile([128, CS + 4], f32, name="xc", tag="xc")
                nc.gpsimd.dma_start(out=xc[0:62, :], in_=xp_d[r0 + 2 : r0 + 126 : 2, :])
                nc.gpsimd.dma_start(out=xc[64:126, :], in_=xp_d[r0 + 3 : r0 + 127 : 2, :])
                return X, xc

            pending = []
            loaded = {k: issue_loads(r0s[k]) for k in range(min(LOAD_AHEAD + 1, len(r0s)))}
            for j, r0 in enumerate(r0s):
                X, xc = loaded.pop(j)
                if j + LOAD_AHEAD + 1 < len(r0s):
                    loaded[j + LOAD_AHEAD + 1] = issue_loads(r0s[j + LOAD_AHEAD + 1])

                psums = []
                for g in range(4):
                    ps = ppool.tile([128, NC2], f32, name=f"ps{g}", tag=f"ps{g}")
                    coff = 0 if g < 2 else 1
                    for dj in range(5):
                        mov = X[:, coff + dj : coff + dj + CS - 1 : 2]
                        nc.tensor.matmul(
                            ps[:],
                            lhsT=wt[:, (g * 5 + dj) * 128 : (g * 5 + dj + 1) * 128],
                            rhs=mov,
                            start=(dj == 0),
                            stop=(dj == 4),
                        )
                    psums.append(ps)
                E1, E2, O1, O2 = psums

                asm = apool.tile([128, CS * 3], f32, name="asm", tag="asm")

                def clip(o, i):
                    nc.vector.tensor_scalar(o, i, 1.0, 0.0, op0=mn, op1=mx)

                L = CS * 3
                clip(asm[0:62, 1:L:6], E1[0:62, :])      # G @ (e,e)
                clip(asm[64:126, 0:L:6], E1[64:126, :])  # R @ (o,e)
                clip(asm[0:128, 2:L:6], E2[0:128, :])    # B @ (e,e)+(o,e)
                clip(asm[0:128, 3:L:6], O1[0:128, :])    # R @ (e,o)+(o,o)
                clip(asm[0:62, 5:L:6], O2[0:62, :])      # B @ (e,o)
                clip(asm[64:126, 4:L:6], O2[64:126, :])  # G @ (o,o)

                # x passthrough (no clip needed: x in [0,1))
                nc.scalar.activation(asm[0:62, 0:L:6], xc[0:62, 2 : 2 + CS : 2], copy_f)
                nc.scalar.activation(asm[64:126, 1:L:6], xc[64:126, 2 : 2 + CS : 2], copy_f)
                nc.gpsimd.tensor_copy(asm[0:62, 4:L:6], xc[0:62, 3 : 3 + CS : 2])
                nc.gpsimd.tensor_copy(asm[64:126, 5:L:6], xc[64:126, 3 : 3 + CS : 2])

                pending.append((r0, asm))
                if len(pending) > STORE_SKEW:
                    store(*pending.pop(0))
            for item in pending:
                store(*item)
    nc.compile()
    return nc


def _get_program():
    global _PROGRAM
    if _PROGRAM is None:
        _PROGRAM = _build_program()
    return _PROGRAM


def _build_stationary(kern):
    """kern: [4,5,5] f32 -> W [128, 20*128] f32 (SBUF layout, lhsT per slice)."""
    groups = [(0, 2), (3, 1), (1, 3), (2, 0)]  # (even-row kernel, odd-row kernel)
    Wm = np.zeros((20, 128, 128), np.float32)
    t = np.arange(62)
    for g, (ka, kb) in enumerate(groups):
        for dj in range(5):
            Wq = Wm[g * 5 + dj]
            for di in range(5):
                # X row order is natural: partition p = xp row r0+p
                Wq[2 * t + di, t] += kern[ka, di, dj]          # even out rows
                Wq[2 * t + 1 + di, 64 + t] += kern[kb, di, dj]  # odd out rows
    # [20,128p,128m] -> [128p, 20*128]
    return np.ascontiguousarray(Wm.transpose(1, 0, 2).reshape(128, 20 * 128))


def kernel(x, kernels, _trace=False):
    from concourse.bass_utils import run_bass_kernel_spmd

    x = np.asarray(x, dtype=np.float32)
    kern = np.asarray(kernels, dtype=np.float32).reshape(4, 5, 5)
    wst = _build_stationary(kern)
    xpad = np.pad(x, 2, mode="reflect")

    in_maps = []
    for c in range(NCORES):
        shard = np.ascontiguousarray(xpad[:, c * CS : c * CS + CS + 4])
        in_maps.append({"xp": shard, "wst": wst})

    nc = _get_program()
    res = run_bass_kernel_spmd(nc, in_maps, list(range(NCORES)), trace=_trace)
    out = np.concatenate(
        [res.results[c]["out"].reshape(H, CS, 3) for c in range(NCORES)], axis=1
    )
    if _trace:
        return out, res
    return out


# revision 16
# speedup vs baseline: 3.7222x; 2.1751x over previous
"""Malvar-He-Cutler demosaic on 8 Trainium2 NeuronCores.

Strategy (W-sharding, all ops local per core):
  - Full input x [4096, 6144] f32 is reflect-padded on host and column-
    sharded into 8 slices of 768 cols (+2 halo each side) -> xp [4100, 772].
  - Per core, row tiles of 124 output rows. Input tile X [128, 772] is
    loaded parity-permuted (partitions 0-63 = even image rows, 64-127 =
    odd rows) by two strided DMAs; the banded stationary matrices absorb
    the permutation.
  - The 4 needed conv maps (2 per pixel: R/B sites need G-interp + the
    opposite-color kernel, G sites need the two R/B kernels) are computed
    as 4 matmul groups x 5 column-tap passes, accumulating in PSUM.
    Each group packs two 62-row conv maps at partition bases 0 and 64
    (M=128). Moving operand = stride-2 column slice of X (one column
    parity per group), dtype float32r for 1 cycle/row PE throughput.
  - DVE tensor_scalar(min 1.0, max 0.0) copies PSUM -> RGB-interleaved
    assembly buffer (fused clip). The x-passthrough channel values are
    copied by ACT/GPSIMD from a partition-shifted copy of X (engine APs
    require partition base 0/32/64, so a SBUF->SBUF DMA re-bases first).
  - Strided DMAs write even/odd assembled rows to the output shard
    [4096, 768*3]; host concatenates shards along W.
"""

import numpy as np

H, W = 4096, 6144
NCORES = 8
CS = W // NCORES          # 768 cols per core
TILE_R = 124              # output rows per tile
NC2 = CS // 2             # matmul moving free dim (384)

_PROGRAM = None


def _f32r_supported():
    return True


def _build_program(use_f32r=True, h=H, cs=CS):
    from concourse import bacc, mybir, tile

    f32 = mybir.dt.float32
    mmdt = mybir.dt.float32r if use_f32r else mybir.dt.float32
    CS, NC2 = cs, cs // 2  # noqa: shadow module constants intentionally

    nc = bacc.Bacc(None, target_bir_lowering=False, debug=True)
    xp_d = nc.dram_tensor("xp", [h + 4, CS + 4], f32, kind="ExternalInput")
    w_d = nc.dram_tensor("wst", [128, 20 * 128], f32, kind="ExternalInput")
    out_d = nc.dram_tensor("out", [h, CS * 3], f32, kind="ExternalOutput")

    r0s = [TILE_R * i for i in range(h // TILE_R)]
    if r0s[-1] + TILE_R < h:
        r0s.append(h - TILE_R)

    mn, mx = mybir.AluOpType.min, mybir.AluOpType.max
    copy_f = mybir.ActivationFunctionType.Copy

    STORE_SKEW = 2  # store tile i while computing tile i+2

    with tile.TileContext(nc) as tc:
        with tc.tile_pool(name="wpool", bufs=1) as wpool, \
             tc.tile_pool(name="xpool", bufs=4) as xpool, \
             tc.tile_pool(name="cpool", bufs=4) as cpool, \
             tc.tile_pool(name="apool", bufs=STORE_SKEW + 2) as apool, \
             tc.tile_pool(name="ppool", bufs=2, space="PSUM") as ppool:

            wt = wpool.tile([128, 20 * 128], mmdt, name="wt")
            nc.sync.dma_start(out=wt[:], in_=w_d.ap().bitcast(mmdt))

            def store(r0, asm):
                # stores on the ACT HWDGE ring, issued STORE_SKEW tiles late so
                # their semaphore waits are already satisfied at issue time
                if r0 % TILE_R == 0:
                    nc.gpsimd.dma_start(out=out_d[r0 : r0 + TILE_R : 2, :], in_=asm[0:62, :])
                    nc.gpsimd.dma_start(out=out_d[r0 + 1 : r0 + TILE_R : 2, :], in_=asm[64:126, :])
                else:
                    # overlap tile: emit only the rows no earlier tile wrote
                    new0 = (r0s[-2] + TILE_R - r0) // 2  # first new slot
                    nc.gpsimd.dma_start(
                        out=out_d[r0 + 2 * new0 : r0 + TILE_R : 2, :],
                        in_=asm[new0:62, :],
                    )
                    nc.gpsimd.dma_start(
                        out=out_d[r0 + 2 * new0 + 1 : r0 + TILE_R : 2, :],
                        in_=asm[64 + new0 : 126, :],
                    )

            LOAD_AHEAD = 2

            def issue_loads(r0):
                X = xpool.tile([128, CS + 4], mmdt, name="X", tag="X")
                # natural row order: partition p <- xp row r0+p (contiguous).
                # SWDGE (gpsimd) splits one DMA across all 16 SDMA engines;
                # the SP HWDGE ring funnels into only 2 and bottlenecks.
                nc.gpsimd.dma_start(out=X[:], in_=xp_d[r0 : r0 + 128, :].bitcast(mmdt))
                # center rows for the x-passthrough copies, re-read from DRAM
                # (engine APs need partition base 0/32/64, so X can't serve
                # directly). One DMA: dst partitions {0-61, 64-125}.
                xc = cpool.tile([128, CS + 4], f32, name="xc", tag="xc")
                nc.gpsimd.dma_start(out=xc[0:62, :], in_=xp_d[r0 + 2 : r0 + 126 : 2, :])
                nc.gpsimd.dma_start(out=xc[64:126, :], in_=xp_d[r0 + 3 : r0 + 127 : 2, :])
                return X, xc

            pending = []
            loaded = {k: issue_loads(r0s[k]) for k in range(min(LOAD_AHEAD + 1, len(r0s)))}
            for j, r0 in enumerate(r0s):
                X, xc = loaded.pop(j)
                if j + LOAD_AHEAD + 1 < len(r0s):
                    loaded[j + LOAD_AHEAD + 1] = issue_loads(r0s[j + LOAD_AHEAD + 1])

                psums = []
                for g in range(4):
                    ps = ppool.tile([128, NC2], f32, name=f"ps{g}", tag=f"ps{g}")
                    coff = 0 if g < 2 else 1
                    for dj in range(5):
                        mov = X[:, coff + dj : coff + dj + CS - 1 : 2]
                        nc.tensor.matmul(
                            ps[:],
                            lhsT=wt[:, (g * 5 + dj) * 128 : (g * 5 + dj + 1) * 128],
                            rhs=mov,
                            start=(dj == 0),
                            stop=(dj == 4),
                        )
                    psums.append(ps)
                E1, E2, O1, O2 = psums

                asm = apool.tile([128, CS * 3], f32, name="asm", tag="asm")

                def clip(o, i):
                    nc.vector.tensor_scalar(o, i, 1.0, 0.0, op0=mn, op1=mx)

                L = CS * 3
                clip(asm[0:62, 1:L:6], E1[0:62, :])      # G @ (e,e)
                clip(asm[64:126, 0:L:6], E1[64:126, :])  # R @ (o,e)
                clip(asm[0:128, 2:L:6], E2[0:128, :])    # B @ (e,e)+(o,e)
                clip(asm[0:128, 3:L:6], O1[0:128, :])    # R @ (e,o)+(o,o)
                clip(asm[0:62, 5:L:6], O2[0:62, :])      # B @ (e,o)
                clip(asm[64:126, 4:L:6], O2[64:126, :])  # G @ (o,o)

                # x passthrough (no clip needed: x in [0,1))
                nc.scalar.activation(asm[0:62, 0:L:6], xc[0:62, 2 : 2 + CS : 2], copy_f)
                nc.scalar.activation(asm[64:126, 1:L:6], xc[64:126, 2 : 2 + CS : 2], copy_f)
                nc.scalar.activation(asm[0:62, 4:L:6], xc[0:62, 3 : 3 + CS : 2], copy_f)
                nc.scalar.activation(asm[64:126, 5:L:6], xc[64:126, 3 : 3 + CS : 2], copy_f)

                pending.append((r0, asm))
                if len(pending) > STORE_SKEW:
                    store(*pending.pop(0))
            for item in pending:
                store(*item)
    nc.compile()
    return nc


def _get_program():
    global _PROGRAM
    if _PROGRAM is None:
        _PROGRAM = _build_program()
    return _PROGRAM


def _build_stationary(kern):
    """kern: [4,5,5] f32 -> W [128, 20*128] f32 (SBUF layout, lhsT per slice)."""
    groups = [(0, 2), (3, 1), (1, 3), (2, 0)]  # (even-row kernel, odd-row kernel)
    Wm = np.zeros((20, 128, 128), np.float32)
    t = np.arange(62)
    for g, (ka, kb) in enumerate(groups):
        for dj in range(5):
            Wq = Wm[g * 5 + dj]
            for di in range(5):
                # X row order is natural: partition p = xp row r0+p
                Wq[2 * t + di, t] += kern[ka, di, dj]          # even out rows
                Wq[2 * t + 1 + di, 64 + t] += kern[kb, di, dj]  # odd out rows
    # [20,128p,128m] -> [128p, 20*128]
    return np.ascontiguousarray(Wm.transpose(1, 0, 2).reshape(128, 20 * 128))


def kernel(x, kernels, _trace=False):
    from concourse.bass_utils import run_bass_kernel_spmd

    x = np.asarray(x, dtype=np.float32)
    kern = np.asarray(kernels, dtype=np.float32).reshape(4, 5, 5)
    wst = _build_stationary(kern)
    xpad = np.pad(x, 2, mode="reflect")

    in_maps = []
    for c in range(NCORES):
        shard = np.ascontiguousarray(xpad[:, c * CS : c * CS + CS + 4])
        in_maps.append({"xp": shard, "wst": wst})

    nc = _get_program()
    res = run_bass_kernel_spmd(nc, in_maps, list(range(NCORES)), trace=_trace)
    out = np.concatenate(
        [res.results[c]["out"].reshape(H, CS, 3) for c in range(NCORES)], axis=1
    )
    if _trace:
        return out, res
    return out


# revision 17
# speedup vs baseline: 3.7375x; 1.0041x over previous
"""Malvar-He-Cutler demosaic on 8 Trainium2 NeuronCores.

Strategy (W-sharding, all ops local per core):
  - Full input x [4096, 6144] f32 is reflect-padded on host and column-
    sharded into 8 slices of 768 cols (+2 halo each side) -> xp [4100, 772].
  - Per core, row tiles of 124 output rows. Input tile X [128, 772] is
    loaded parity-permuted (partitions 0-63 = even image rows, 64-127 =
    odd rows) by two strided DMAs; the banded stationary matrices absorb
    the permutation.
  - The 4 needed conv maps (2 per pixel: R/B sites need G-interp + the
    opposite-color kernel, G sites need the two R/B kernels) are computed
    as 4 matmul groups x 5 column-tap passes, accumulating in PSUM.
    Each group packs two 62-row conv maps at partition bases 0 and 64
    (M=128). Moving operand = stride-2 column slice of X (one column
    parity per group), dtype float32r for 1 cycle/row PE throughput.
  - DVE tensor_scalar(min 1.0, max 0.0) copies PSUM -> RGB-interleaved
    assembly buffer (fused clip). The x-passthrough channel values are
    copied by ACT/GPSIMD from a partition-shifted copy of X (engine APs
    require partition base 0/32/64, so a SBUF->SBUF DMA re-bases first).
  - Strided DMAs write even/odd assembled rows to the output shard
    [4096, 768*3]; host concatenates shards along W.
"""

import numpy as np

H, W = 4096, 6144
NCORES = 8
CS = W // NCORES          # 768 cols per core
TILE_R = 124              # output rows per tile
NC2 = CS // 2             # matmul moving free dim (384)

_PROGRAM = None


def _f32r_supported():
    return True


def _build_program(use_f32r=True, h=H, cs=CS):
    from concourse import bacc, mybir, tile

    f32 = mybir.dt.float32
    mmdt = mybir.dt.float32r if use_f32r else mybir.dt.float32
    CS, NC2 = cs, cs // 2  # noqa: shadow module constants intentionally

    nc = bacc.Bacc(None, target_bir_lowering=False, debug=True)
    xp_d = nc.dram_tensor("xp", [h + 4, CS + 4], f32, kind="ExternalInput")
    w_d = nc.dram_tensor("wst", [128, 20 * 128], f32, kind="ExternalInput")
    out_d = nc.dram_tensor("out", [h, CS * 3], f32, kind="ExternalOutput")

    r0s = [TILE_R * i for i in range(h // TILE_R)]
    if r0s[-1] + TILE_R < h:
        r0s.append(h - TILE_R)

    mn, mx = mybir.AluOpType.min, mybir.AluOpType.max
    copy_f = mybir.ActivationFunctionType.Copy

    STORE_SKEW = 2  # store tile i while computing tile i+2

    with tile.TileContext(nc) as tc:
        with tc.tile_pool(name="wpool", bufs=1) as wpool, \
             tc.tile_pool(name="xpool", bufs=6) as xpool, \
             tc.tile_pool(name="cpool", bufs=6) as cpool, \
             tc.tile_pool(name="apool", bufs=STORE_SKEW + 2) as apool, \
             tc.tile_pool(name="ppool", bufs=2, space="PSUM") as ppool:

            wt = wpool.tile([128, 20 * 128], mmdt, name="wt")
            nc.sync.dma_start(out=wt[:], in_=w_d.ap().bitcast(mmdt))

            def store(r0, asm):
                # stores on the ACT HWDGE ring, issued STORE_SKEW tiles late so
                # their semaphore waits are already satisfied at issue time
                if r0 % TILE_R == 0:
                    nc.gpsimd.dma_start(out=out_d[r0 : r0 + TILE_R : 2, :], in_=asm[0:62, :])
                    nc.gpsimd.dma_start(out=out_d[r0 + 1 : r0 + TILE_R : 2, :], in_=asm[64:126, :])
                else:
                    # overlap tile: emit only the rows no earlier tile wrote
                    new0 = (r0s[-2] + TILE_R - r0) // 2  # first new slot
                    nc.gpsimd.dma_start(
                        out=out_d[r0 + 2 * new0 : r0 + TILE_R : 2, :],
                        in_=asm[new0:62, :],
                    )
                    nc.gpsimd.dma_start(
                        out=out_d[r0 + 2 * new0 + 1 : r0 + TILE_R : 2, :],
                        in_=asm[64 + new0 : 126, :],
                    )

            LOAD_AHEAD = 4

            def issue_loads(r0):
                X = xpool.tile([128, CS + 4], mmdt, name="X", tag="X")
                # natural row order: partition p <- xp row r0+p (contiguous).
                # SWDGE (gpsimd) splits one DMA across all 16 SDMA engines;
                # the SP HWDGE ring funnels into only 2 and bottlenecks.
                nc.gpsimd.dma_start(out=X[:], in_=xp_d[r0 : r0 + 128, :].bitcast(mmdt))
                # center rows for the x-passthrough copies, re-read from DRAM
                # (engine APs need partition base 0/32/64, so X can't serve
                # directly). One DMA: dst partitions {0-61, 64-125}.
                xc = cpool.tile([128, CS + 4], f32, name="xc", tag="xc")
                nc.gpsimd.dma_start(out=xc[0:62, :], in_=xp_d[r0 + 2 : r0 + 126 : 2, :])
                nc.gpsimd.dma_start(out=xc[64:126, :], in_=xp_d[r0 + 3 : r0 + 127 : 2, :])
                return X, xc

            pending = []
            loaded = {k: issue_loads(r0s[k]) for k in range(min(LOAD_AHEAD + 1, len(r0s)))}
            for j, r0 in enumerate(r0s):
                X, xc = loaded.pop(j)
                if j + LOAD_AHEAD + 1 < len(r0s):
                    loaded[j + LOAD_AHEAD + 1] = issue_loads(r0s[j + LOAD_AHEAD + 1])

                psums = []
                for g in range(4):
                    ps = ppool.tile([128, NC2], f32, name=f"ps{g}", tag=f"ps{g}")
                    coff = 0 if g < 2 else 1
                    for dj in range(5):
                        mov = X[:, coff + dj : coff + dj + CS - 1 : 2]
                        nc.tensor.matmul(
                            ps[:],
                            lhsT=wt[:, (g * 5 + dj) * 128 : (g * 5 + dj + 1) * 128],
                            rhs=mov,
                            start=(dj == 0),
                            stop=(dj == 4),
                        )
                    psums.append(ps)
                E1, E2, O1, O2 = psums

                asm = apool.tile([128, CS * 3], f32, name="asm", tag="asm")

                def clip(o, i):
                    nc.vector.tensor_scalar(o, i, 1.0, 0.0, op0=mn, op1=mx)

                L = CS * 3
                clip(asm[0:62, 1:L:6], E1[0:62, :])      # G @ (e,e)
                clip(asm[64:126, 0:L:6], E1[64:126, :])  # R @ (o,e)
                clip(asm[0:128, 2:L:6], E2[0:128, :])    # B @ (e,e)+(o,e)
                clip(asm[0:128, 3:L:6], O1[0:128, :])    # R @ (e,o)+(o,o)
                clip(asm[0:62, 5:L:6], O2[0:62, :])      # B @ (e,o)
                clip(asm[64:126, 4:L:6], O2[64:126, :])  # G @ (o,o)

                # x passthrough (no clip needed: x in [0,1))
                nc.scalar.activation(asm[0:62, 0:L:6], xc[0:62, 2 : 2 + CS : 2], copy_f)
                nc.scalar.activation(asm[64:126, 1:L:6], xc[64:126, 2 : 2 + CS : 2], copy_f)
                nc.scalar.activation(asm[0:62, 4:L:6], xc[0:62, 3 : 3 + CS : 2], copy_f)
                nc.scalar.activation(asm[64:126, 5:L:6], xc[64:126, 3 : 3 + CS : 2], copy_f)

                pending.append((r0, asm))
                if len(pending) > STORE_SKEW:
                    store(*pending.pop(0))
            for item in pending:
                store(*item)
    nc.compile()
    return nc


def _get_program():
    global _PROGRAM
    if _PROGRAM is None:
        _PROGRAM = _build_program()
    return _PROGRAM


def _build_stationary(kern):
    """kern: [4,5,5] f32 -> W [128, 20*128] f32 (SBUF layout, lhsT per slice)."""
    groups = [(0, 2), (3, 1), (1, 3), (2, 0)]  # (even-row kernel, odd-row kernel)
    Wm = np.zeros((20, 128, 128), np.float32)
    t = np.arange(62)
    for g, (ka, kb) in enumerate(groups):
        for dj in range(5):
            Wq = Wm[g * 5 + dj]
            for di in range(5):
                # X row order is natural: partition p = xp row r0+p
                Wq[2 * t + di, t] += kern[ka, di, dj]          # even out rows
                Wq[2 * t + 1 + di, 64 + t] += kern[kb, di, dj]  # odd out rows
    # [20,128p,128m] -> [128p, 20*128]
    return np.ascontiguousarray(Wm.transpose(1, 0, 2).reshape(128, 20 * 128))


def kernel(x, kernels, _trace=False):
    from concourse.bass_utils import run_bass_kernel_spmd

    x = np.asarray(x, dtype=np.float32)
    kern = np.asarray(kernels, dtype=np.float32).reshape(4, 5, 5)
    wst = _build_stationary(kern)
    xpad = np.pad(x, 2, mode="reflect")

    in_maps = []
    for c in range(NCORES):
        shard = np.ascontiguousarray(xpad[:, c * CS : c * CS + CS + 4])
        in_maps.append({"xp": shard, "wst": wst})

    nc = _get_program()
    res = run_bass_kernel_spmd(nc, in_maps, list(range(NCORES)), trace=_trace)
    out = np.concatenate(
        [res.results[c]["out"].reshape(H, CS, 3) for c in range(NCORES)], axis=1
    )
    if _trace:
        return out, res
    return out


# revision 18
# speedup vs baseline: 4.0344x; 1.0794x over previous
"""Malvar-He-Cutler demosaic on 8 Trainium2 NeuronCores.

Strategy (W-sharding, all ops local per core):
  - Full input x [4096, 6144] f32 is reflect-padded on host and column-
    sharded into 8 slices of 768 cols (+2 halo each side) -> xp [4100, 772].
  - Per core, row tiles of 124 output rows. Input tile X [128, 772] is
    loaded parity-permuted (partitions 0-63 = even image rows, 64-127 =
    odd rows) by two strided DMAs; the banded stationary matrices absorb
    the permutation.
  - The 4 needed conv maps (2 per pixel: R/B sites need G-interp + the
    opposite-color kernel, G sites need the two R/B kernels) are computed
    as 4 matmul groups x 5 column-tap passes, accumulating in PSUM.
    Each group packs two 62-row conv maps at partition bases 0 and 64
    (M=128). Moving operand = stride-2 column slice of X (one column
    parity per group), dtype float32r for 1 cycle/row PE throughput.
  - DVE tensor_scalar(min 1.0, max 0.0) copies PSUM -> RGB-interleaved
    assembly buffer (fused clip). The x-passthrough channel values are
    copied by ACT/GPSIMD from a partition-shifted copy of X (engine APs
    require partition base 0/32/64, so a SBUF->SBUF DMA re-bases first).
  - Strided DMAs write even/odd assembled rows to the output shard
    [4096, 768*3]; host concatenates shards along W.
"""

import numpy as np

H, W = 4096, 6144
NCORES = 8
CS = W // NCORES          # 768 cols per core
TILE_R = 124              # output rows per tile
NC2 = CS // 2             # matmul moving free dim (384)

_PROGRAM = None


def _f32r_supported():
    return True


def _build_program(use_f32r=True, h=H, cs=CS):
    from concourse import bacc, mybir, tile

    f32 = mybir.dt.float32
    mmdt = mybir.dt.float32r if use_f32r else mybir.dt.float32
    CS, NC2 = cs, cs // 2  # noqa: shadow module constants intentionally

    nc = bacc.Bacc(None, target_bir_lowering=False, debug=True)
    xp_d = nc.dram_tensor("xp", [h + 4, CS + 4], f32, kind="ExternalInput")
    w_d = nc.dram_tensor("wst", [128, 22 * 128], f32, kind="ExternalInput")
    out_d = nc.dram_tensor("out", [h, CS * 3], f32, kind="ExternalOutput")

    r0s = [TILE_R * i for i in range(h // TILE_R)]
    if r0s[-1] + TILE_R < h:
        r0s.append(h - TILE_R)

    mn, mx = mybir.AluOpType.min, mybir.AluOpType.max
    copy_f = mybir.ActivationFunctionType.Copy

    STORE_SKEW = 2  # store tile i while computing tile i+2

    with tile.TileContext(nc) as tc:
        with tc.tile_pool(name="wpool", bufs=1) as wpool, \
             tc.tile_pool(name="xpool", bufs=6) as xpool, \
             tc.tile_pool(name="apool", bufs=STORE_SKEW + 2) as apool, \
             tc.tile_pool(name="ppool", bufs=1, space="PSUM") as ppool:

            wt = wpool.tile([128, 22 * 128], mmdt, name="wt")
            nc.sync.dma_start(out=wt[:], in_=w_d.ap().bitcast(mmdt))

            def store(r0, asm):
                # stores on the ACT HWDGE ring, issued STORE_SKEW tiles late so
                # their semaphore waits are already satisfied at issue time
                if r0 % TILE_R == 0:
                    nc.gpsimd.dma_start(out=out_d[r0 : r0 + TILE_R : 2, :], in_=asm[0:62, :])
                    nc.gpsimd.dma_start(out=out_d[r0 + 1 : r0 + TILE_R : 2, :], in_=asm[64:126, :])
                else:
                    # overlap tile: emit only the rows no earlier tile wrote
                    new0 = (r0s[-2] + TILE_R - r0) // 2  # first new slot
                    nc.gpsimd.dma_start(
                        out=out_d[r0 + 2 * new0 : r0 + TILE_R : 2, :],
                        in_=asm[new0:62, :],
                    )
                    nc.gpsimd.dma_start(
                        out=out_d[r0 + 2 * new0 + 1 : r0 + TILE_R : 2, :],
                        in_=asm[64 + new0 : 126, :],
                    )

            LOAD_AHEAD = 4

            def issue_loads(r0):
                X = xpool.tile([128, CS + 4], mmdt, name="X", tag="X")
                # natural row order: partition p <- xp row r0+p (contiguous).
                # SWDGE (gpsimd) splits one DMA across all 16 SDMA engines;
                # the SP HWDGE ring funnels into only 2 and bottlenecks.
                nc.gpsimd.dma_start(out=X[:], in_=xp_d[r0 : r0 + 128, :].bitcast(mmdt))
                return X

            pending = []
            loaded = {k: issue_loads(r0s[k]) for k in range(min(LOAD_AHEAD + 1, len(r0s)))}
            for j, r0 in enumerate(r0s):
                X = loaded.pop(j)
                if j + LOAD_AHEAD + 1 < len(r0s):
                    loaded[j + LOAD_AHEAD + 1] = issue_loads(r0s[j + LOAD_AHEAD + 1])

                psums = []
                for g in range(4):
                    ps = ppool.tile([128, NC2], f32, name=f"ps{g}", tag=f"ps{g}")
                    coff = 0 if g < 2 else 1
                    for dj in range(5):
                        mov = X[:, coff + dj : coff + dj + CS - 1 : 2]
                        nc.tensor.matmul(
                            ps[:],
                            lhsT=wt[:, (g * 5 + dj) * 128 : (g * 5 + dj + 1) * 128],
                            rhs=mov,
                            start=(dj == 0),
                            stop=(dj == 4),
                        )
                    psums.append(ps)
                for g, q in ((4, 20), (5, 21)):  # identity taps: E3 (even cols), O3 (odd cols)
                    ps = ppool.tile([128, NC2], f32, name=f"ps{g}", tag=f"ps{g}")
                    coff = 2 if g == 4 else 3
                    nc.tensor.matmul(
                        ps[:],
                        lhsT=wt[:, q * 128 : (q + 1) * 128],
                        rhs=X[:, coff : coff + CS - 1 : 2],
                        start=True,
                        stop=True,
                    )
                    psums.append(ps)
                E1, E2, O1, O2, E3, O3 = psums

                asm = apool.tile([128, CS * 3], f32, name="asm", tag="asm")

                def clip(o, i):
                    nc.vector.tensor_scalar(o, i, 1.0, 0.0, op0=mn, op1=mx)

                L = CS * 3
                clip(asm[0:62, 1:L:6], E1[0:62, :])      # G @ (e,e)
                clip(asm[64:126, 0:L:6], E1[64:126, :])  # R @ (o,e)
                clip(asm[0:128, 2:L:6], E2[0:128, :])    # B @ (e,e)+(o,e)
                clip(asm[0:128, 3:L:6], O1[0:128, :])    # R @ (e,o)+(o,o)
                clip(asm[0:62, 5:L:6], O2[0:62, :])      # B @ (e,o)
                clip(asm[64:126, 4:L:6], O2[64:126, :])  # G @ (o,o)

                # x passthrough via PE identity taps (no clip: x in [0,1))
                nc.scalar.activation(asm[0:62, 0:L:6], E3[0:62, :], copy_f)    # R @ (e,e)
                nc.scalar.activation(asm[64:126, 1:L:6], E3[64:126, :], copy_f)  # G @ (o,e)
                nc.scalar.activation(asm[0:62, 4:L:6], O3[0:62, :], copy_f)    # G @ (e,o)
                nc.scalar.activation(asm[64:126, 5:L:6], O3[64:126, :], copy_f)  # B @ (o,o)

                pending.append((r0, asm))
                if len(pending) > STORE_SKEW:
                    store(*pending.pop(0))
            for item in pending:
                store(*item)
    nc.compile()
    return nc


def _get_program():
    global _PROGRAM
    if _PROGRAM is None:
        _PROGRAM = _build_program()
    return _PROGRAM


def _build_stationary(kern):
    """kern: [4,5,5] f32 -> W [128, 20*128] f32 (SBUF layout, lhsT per slice)."""
    groups = [(0, 2), (3, 1), (1, 3), (2, 0)]  # (even-row kernel, odd-row kernel)
    Wm = np.zeros((22, 128, 128), np.float32)
    t = np.arange(62)
    for g, (ka, kb) in enumerate(groups):
        for dj in range(5):
            Wq = Wm[g * 5 + dj]
            for di in range(5):
                # X row order is natural: partition p = xp row r0+p
                Wq[2 * t + di, t] += kern[ka, di, dj]          # even out rows
                Wq[2 * t + 1 + di, 64 + t] += kern[kb, di, dj]  # odd out rows
    for t in range(62):  # identity taps (center of the 5x5 window)
        Wm[20, 2 * t + 2, t] = 1.0       # x @ (e,e) site, even cols
        Wm[20, 2 * t + 3, 64 + t] = 1.0  # x @ (o,e) site, even cols
        Wm[21, 2 * t + 2, t] = 1.0       # x @ (e,o) site, odd cols
        Wm[21, 2 * t + 3, 64 + t] = 1.0  # x @ (o,o) site, odd cols
    # [22,128p,128m] -> [128p, 22*128]
    return np.ascontiguousarray(Wm.transpose(1, 0, 2).reshape(128, 22 * 128))


def kernel(x, kernels, _trace=False):
    from concourse.bass_utils import run_bass_kernel_spmd

    x = np.asarray(x, dtype=np.float32)
    kern = np.asarray(kernels, dtype=np.float32).reshape(4, 5, 5)
    wst = _build_stationary(kern)
    xpad = np.pad(x, 2, mode="reflect")

    in_maps = []
    for c in range(NCORES):
        shard = np.ascontiguousarray(xpad[:, c * CS : c * CS + CS + 4])
        in_maps.append({"xp": shard, "wst": wst})

    nc = _get_program()
    res = run_bass_kernel_spmd(nc, in_maps, list(range(NCORES)), trace=_trace)
    out = np.concatenate(
        [res.results[c]["out"].reshape(H, CS, 3) for c in range(NCORES)], axis=1
    )
    if _trace:
        return out, res
    return out


# revision 19
# speedup vs baseline: 4.6491x; 1.1524x over previous
"""Malvar-He-Cutler demosaic on 8 Trainium2 NeuronCores.

Strategy (W-sharding, all ops local per core):
  - Full input x [4096, 6144] f32 is reflect-padded on host and column-
    sharded into 8 slices of 768 cols (+2 halo each side) -> xp [4100, 772].
  - Per core, row tiles of 124 output rows. Input tile X [128, 772] is
    loaded parity-permuted (partitions 0-63 = even image rows, 64-127 =
    odd rows) by two strided DMAs; the banded stationary matrices absorb
    the permutation.
  - The 4 needed conv maps (2 per pixel: R/B sites need G-interp + the
    opposite-color kernel, G sites need the two R/B kernels) are computed
    as 4 matmul groups x 5 column-tap passes, accumulating in PSUM.
    Each group packs two 62-row conv maps at partition bases 0 and 64
    (M=128). Moving operand = stride-2 column slice of X (one column
    parity per group), dtype float32r for 1 cycle/row PE throughput.
  - DVE tensor_scalar(min 1.0, max 0.0) copies PSUM -> RGB-interleaved
    assembly buffer (fused clip). The x-passthrough channel values are
    copied by ACT/GPSIMD from a partition-shifted copy of X (engine APs
    require partition base 0/32/64, so a SBUF->SBUF DMA re-bases first).
  - Strided DMAs write even/odd assembled rows to the output shard
    [4096, 768*3]; host concatenates shards along W.
"""

import numpy as np

H, W = 4096, 6144
NCORES = 8
CS = W // NCORES          # 768 cols per core
TILE_R = 124              # output rows per tile
NC2 = CS // 2             # matmul moving free dim (384)

_PROGRAM = None


def _f32r_supported():
    return True


def _build_program(use_f32r=True, h=H, cs=CS):
    from concourse import bacc, mybir, tile

    f32 = mybir.dt.float32
    mmdt = mybir.dt.float32r if use_f32r else mybir.dt.float32
    CS, NC2 = cs, cs // 2  # noqa: shadow module constants intentionally

    nc = bacc.Bacc(None, target_bir_lowering=False, debug=True)
    xp_d = nc.dram_tensor("xp", [h + 4, CS + 4], f32, kind="ExternalInput")
    w_d = nc.dram_tensor("wst", [128, 22 * 128], f32, kind="ExternalInput")
    out_d = nc.dram_tensor("out", [h, CS * 3], f32, kind="ExternalOutput")

    r0s = [TILE_R * i for i in range(h // TILE_R)]
    if r0s[-1] + TILE_R < h:
        r0s.append(h - TILE_R)

    mn, mx = mybir.AluOpType.min, mybir.AluOpType.max
    copy_f = mybir.ActivationFunctionType.Copy

    STORE_SKEW = 2  # store tile i while computing tile i+2

    with tile.TileContext(nc) as tc:
        with tc.tile_pool(name="wpool", bufs=1) as wpool, \
             tc.tile_pool(name="xpool", bufs=6) as xpool, \
             tc.tile_pool(name="apool", bufs=STORE_SKEW + 2) as apool, \
             tc.tile_pool(name="ppool", bufs=1, space="PSUM") as ppool:

            wt = wpool.tile([128, 22 * 128], mmdt, name="wt")
            nc.sync.dma_start(out=wt[:], in_=w_d.ap().bitcast(mmdt))

            def store(r0, asm):
                # stores on the ACT HWDGE ring, issued STORE_SKEW tiles late so
                # their semaphore waits are already satisfied at issue time
                if r0 % TILE_R == 0:
                    nc.gpsimd.dma_start(out=out_d[r0 : r0 + TILE_R : 2, :], in_=asm[0:62, :])
                    nc.gpsimd.dma_start(out=out_d[r0 + 1 : r0 + TILE_R : 2, :], in_=asm[64:126, :])
                else:
                    # overlap tile: emit only the rows no earlier tile wrote
                    new0 = (r0s[-2] + TILE_R - r0) // 2  # first new slot
                    nc.gpsimd.dma_start(
                        out=out_d[r0 + 2 * new0 : r0 + TILE_R : 2, :],
                        in_=asm[new0:62, :],
                    )
                    nc.gpsimd.dma_start(
                        out=out_d[r0 + 2 * new0 + 1 : r0 + TILE_R : 2, :],
                        in_=asm[64 + new0 : 126, :],
                    )

            LOAD_AHEAD = 4

            def issue_loads(r0):
                X = xpool.tile([128, CS + 4], mmdt, name="X", tag="X")
                # natural row order: partition p <- xp row r0+p (contiguous).
                # SWDGE (gpsimd) splits one DMA across all 16 SDMA engines;
                # the SP HWDGE ring funnels into only 2 and bottlenecks.
                nc.gpsimd.dma_start(out=X[:], in_=xp_d[r0 : r0 + 128, :].bitcast(mmdt))
                return X

            pending = []
            loaded = {k: issue_loads(r0s[k]) for k in range(min(LOAD_AHEAD + 1, len(r0s)))}
            for j, r0 in enumerate(r0s):
                X = loaded.pop(j)
                if j + LOAD_AHEAD + 1 < len(r0s):
                    loaded[j + LOAD_AHEAD + 1] = issue_loads(r0s[j + LOAD_AHEAD + 1])

                psums = []
                for g in range(4):
                    ps = ppool.tile([128, NC2], f32, name=f"ps{g}", tag=f"ps{g}",
                                    bufs=2 if g < 2 else 1)
                    coff = 0 if g < 2 else 1
                    for dj in range(5):
                        mov = X[:, coff + dj : coff + dj + CS - 1 : 2]
                        nc.tensor.matmul(
                            ps[:],
                            lhsT=wt[:, (g * 5 + dj) * 128 : (g * 5 + dj + 1) * 128],
                            rhs=mov,
                            start=(dj == 0),
                            stop=(dj == 4),
                        )
                    psums.append(ps)
                for g, q in ((4, 20), (5, 21)):  # identity taps: E3 (even cols), O3 (odd cols)
                    ps = ppool.tile([128, NC2], f32, name=f"ps{g}", tag=f"ps{g}")
                    coff = 2 if g == 4 else 3
                    nc.tensor.matmul(
                        ps[:],
                        lhsT=wt[:, q * 128 : (q + 1) * 128],
                        rhs=X[:, coff : coff + CS - 1 : 2],
                        start=True,
                        stop=True,
                    )
                    psums.append(ps)
                E1, E2, O1, O2, E3, O3 = psums

                asm = apool.tile([128, CS * 3], f32, name="asm", tag="asm")

                def clip(o, i):
                    nc.vector.tensor_scalar(o, i, 1.0, 0.0, op0=mn, op1=mx)

                L = CS * 3
                clip(asm[0:62, 1:L:6], E1[0:62, :])      # G @ (e,e)
                clip(asm[64:126, 0:L:6], E1[64:126, :])  # R @ (o,e)
                clip(asm[0:128, 2:L:6], E2[0:128, :])    # B @ (e,e)+(o,e)
                clip(asm[0:128, 3:L:6], O1[0:128, :])    # R @ (e,o)+(o,o)
                clip(asm[0:62, 5:L:6], O2[0:62, :])      # B @ (e,o)
                clip(asm[64:126, 4:L:6], O2[64:126, :])  # G @ (o,o)

                # x passthrough via PE identity taps (no clip: x in [0,1))
                nc.scalar.activation(asm[0:62, 0:L:6], E3[0:62, :], copy_f)    # R @ (e,e)
                nc.scalar.activation(asm[64:126, 1:L:6], E3[64:126, :], copy_f)  # G @ (o,e)
                nc.scalar.activation(asm[0:62, 4:L:6], O3[0:62, :], copy_f)    # G @ (e,o)
                nc.scalar.activation(asm[64:126, 5:L:6], O3[64:126, :], copy_f)  # B @ (o,o)

                pending.append((r0, asm))
                if len(pending) > STORE_SKEW:
                    store(*pending.pop(0))
            for item in pending:
                store(*item)
    nc.compile()
    return nc


def _get_program():
    global _PROGRAM
    if _PROGRAM is None:
        _PROGRAM = _build_program()
    return _PROGRAM


def _build_stationary(kern):
    """kern: [4,5,5] f32 -> W [128, 20*128] f32 (SBUF layout, lhsT per slice)."""
    groups = [(0, 2), (3, 1), (1, 3), (2, 0)]  # (even-row kernel, odd-row kernel)
    Wm = np.zeros((22, 128, 128), np.float32)
    t = np.arange(62)
    for g, (ka, kb) in enumerate(groups):
        for dj in range(5):
            Wq = Wm[g * 5 + dj]
            for di in range(5):
                # X row order is natural: partition p = xp row r0+p
                Wq[2 * t + di, t] += kern[ka, di, dj]          # even out rows
                Wq[2 * t + 1 + di, 64 + t] += kern[kb, di, dj]  # odd out rows
    for t in range(62):  # identity taps (center of the 5x5 window)
        Wm[20, 2 * t + 2, t] = 1.0       # x @ (e,e) site, even cols
        Wm[20, 2 * t + 3, 64 + t] = 1.0  # x @ (o,e) site, even cols
        Wm[21, 2 * t + 2, t] = 1.0       # x @ (e,o) site, odd cols
        Wm[21, 2 * t + 3, 64 + t] = 1.0  # x @ (o,o) site, odd cols
    # [22,128p,128m] -> [128p, 22*128]
    return np.ascontiguousarray(Wm.transpose(1, 0, 2).reshape(128, 22 * 128))


def kernel(x, kernels, _trace=False):
    from concourse.bass_utils import run_bass_kernel_spmd

    x = np.asarray(x, dtype=np.float32)
    kern = np.asarray(kernels, dtype=np.float32).reshape(4, 5, 5)
    wst = _build_stationary(kern)
    xpad = np.pad(x, 2, mode="reflect")

    in_maps = []
    for c in range(NCORES):
        shard = np.ascontiguousarray(xpad[:, c * CS : c * CS + CS + 4])
        in_maps.append({"xp": shard, "wst": wst})

    nc = _get_program()
    res = run_bass_kernel_spmd(nc, in_maps, list(range(NCORES)), trace=_trace)
    out = np.concatenate(
        [res.results[c]["out"].reshape(H, CS, 3) for c in range(NCORES)], axis=1
    )
    if _trace:
        return out, res
    return out
